# revision 1
# baseline (speedup 1.0000x reference)
"""Trainium2 Bass kernel for nn_BetweenClusterFC.

Computes out[e] = (emb_1[f[e]] @ W1 + b1) . (emb_2[t[e]] @ W2 + b2)
for E = 1.6M edges over N = 100k nodes, D_IN = 256, D_OUT = 128.

Strategy (8 NeuronCores, SPMD, full inputs in / full output out):
  - Nodes are split into 8 blocks of 12500.  Edges are assigned to cores by a
    (from-block-group, to-block-group) 4x2 rectangle: core c=(a,b) handles
    edges with from-node in blocks [4a..4a+3] and to-node in blocks
    [2b..2b+1].  Uniform (~200k edges/core), and each core only needs
    projections for 4 from-blocks + 2 to-blocks (75k nodes) instead of a
    fully replicated 200k -> far less HBM traffic.
  - Each core projects its 6 blocks (p = emb @ W + b) on the PE from
    host-pre-transposed embedding shards, writing p1/p2 tables to local DRAM.
  - Edges are bucketed host-side by (local from-block, local to-block) into
    8 buckets/core; per bucket both endpoint rows are fetched with the SWDGE
    dma_gather instruction (int16 local indices, 512B rows), then a DVE
    multiply + reduce produces the per-edge dot products.
  - The host applies the inverse edge permutation to assemble the output.

Written in raw Bass (explicit semaphores) — the Tile layer's generated sync
exceeds this toolchain's per-instruction wait-slot limits.
"""

import contextlib
import math

import numpy as np

import concourse.bass as bass
import concourse.mybir as mybir

# ---------------------------------------------------------------- constants
N_NODES = 100_000
D_IN = 256
D_OUT = 128
N_EDGES = 1_600_000
N_CORES = 8

NB = 12_500          # nodes per block
NBP = 12_544         # padded block rows (98 * 128)
NFB = 4              # from-blocks per core
NTB = 2              # to-blocks per core
NBUCKET = NFB * NTB  # 8 buckets per core

CAP = 26_624         # padded edge capacity per bucket (mean 25k, +10 sigma)
CALLS = [1024] * 26          # dma_gather call sizes (HW limit: <=1024 idxs/call)
assert sum(CALLS) == CAP
CALL_COLS = [g // 16 for g in CALLS]   # idx columns per call (wrapped by 16)
CALL_SLOTS = [g // 128 for g in CALLS]  # result slots per call
SLOT_TOT = CAP // 128                  # 208 result columns per bucket
COLS_PER_BUCKET = CAP // 16            # 1664 idx columns per bucket
IDX_COLS = NBUCKET * COLS_PER_BUCKET   # 13312

P1_ROWS = NFB * NBP  # 50176
P2_ROWS = NTB * NBP  # 25088

TILES1 = P1_ROWS // 128    # 392 node-tiles, table 1
TILES2 = P2_ROWS // 128    # 196 node-tiles, table 2
GROUPS1 = TILES1 // 4      # 98 psum groups
GROUPS2 = TILES2 // 4      # 49
NGROUP = GROUPS1 + GROUPS2  # 147
CHUNK_T = 14               # node-tiles per embT load chunk
NCH1 = TILES1 // CHUNK_T   # 28 chunks
NCH2 = TILES2 // CHUNK_T   # 14
NCHUNK = NCH1 + NCH2       # 42
EMB_COLS = CHUNK_T * 128   # 1792

NCALL = NBUCKET * len(CALLS)  # 56 gather calls per side

F32 = mybir.dt.float32
I16 = mybir.dt.int16
AX = mybir.AxisListType


# Processing order: p2 groups first, then p1 -> p-blocks finish progressively
# (p2b0@25, p2b1@49, p1b0@74, p1b1@98, p1b2@123, p1b3@147 positions), letting
# fi-major gather buckets start while later p1 blocks still project.
GSEQ = list(range(GROUPS1, NGROUP)) + list(range(GROUPS1))
CSEQ = list(range(NCH1, NCHUNK)) + list(range(NCH1))
CPOS = {cid: q for q, cid in enumerate(CSEQ)}
# pool gate positions: bucket group fi ready after this many processed groups
FI_READY = [49 + math.ceil(24.5 * (fi + 1)) for fi in range(NFB)]  # 74,98,123,147
INTERLEAVE_Q = 76  # start draining gather calls into the DVE stream here


def _chunk_of_tile(tg):
    """global tile index -> (global chunk id, table, local col0)."""
    if tg < TILES1:
        c = tg // CHUNK_T
        return c, 0, (tg % CHUNK_T) * 128
    t2 = tg - TILES1
    c = NCH1 + t2 // CHUNK_T
    return c, 1, (t2 % CHUNK_T) * 128


def _chunk_last_tile(c):
    """global chunk id -> global index of its last tile."""
    if c < NCH1:
        return (c + 1) * CHUNK_T - 1
    return TILES1 + (c - NCH1 + 1) * CHUNK_T - 1


def _chunk_src(c):
    """global chunk id -> (table, col0)."""
    if c < NCH1:
        return 0, c * EMB_COLS
    return 1, (c - NCH1) * EMB_COLS


# ---------------------------------------------------------------- device code
def build_bass(phase="all"):
    """phase: "all" | "proj" (p tables as outputs, no gather) |
    "gather" (p tables as inputs, no projection).  Non-"all" modes exist for
    hardware bring-up/debugging."""
    nc = bass.Bass()

    e1t = nc.dram_tensor("e1t", [D_IN, P1_ROWS], F32, kind="ExternalInput")
    e2t = nc.dram_tensor("e2t", [D_IN, P2_ROWS], F32, kind="ExternalInput")
    w1 = nc.dram_tensor("w1", [D_IN, D_OUT], F32, kind="ExternalInput")
    w2 = nc.dram_tensor("w2", [D_IN, D_OUT], F32, kind="ExternalInput")
    b1f = nc.dram_tensor("b1f", [128, 512], F32, kind="ExternalInput")
    b2f = nc.dram_tensor("b2f", [128, 512], F32, kind="ExternalInput")
    idxa = nc.dram_tensor("idxa", [128, IDX_COLS], I16, kind="ExternalInput")
    idxb = nc.dram_tensor("idxb", [128, IDX_COLS], I16, kind="ExternalInput")
    res = nc.dram_tensor("res", [NBUCKET, 128, SLOT_TOT], F32, kind="ExternalOutput")

    pkind = {"all": "Internal", "proj": "ExternalOutput", "gather": "ExternalInput"}[phase]
    p1d = nc.dram_tensor("p1d", [P1_ROWS, D_OUT], F32, kind=pkind)
    p2d = nc.dram_tensor("p2d", [P2_ROWS, D_OUT], F32, kind=pkind)
    pdst = (p1d, p2d)
    do_proj = phase in ("all", "proj")
    do_gather = phase in ("all", "gather")

    st = contextlib.ExitStack()
    with st:
        sb = lambda nm, shape, dt=F32: st.enter_context(nc.sbuf_tensor(nm, shape, dt))
        sem = lambda nm: st.enter_context(nc.semaphore(name=nm))

        w1c = sb("w1c", [128, 256])
        w2c = sb("w2c", [128, 256])
        bt = (sb("bt1", [128, 512]), sb("bt2", [128, 512]))
        idxt = (sb("idxta", [128, IDX_COLS], I16), sb("idxtb", [128, IDX_COLS], I16))
        et = [[sb(f"et_{p}_{k}", [128, EMB_COLS]) for k in range(2)]
              for p in range(2)]  # [parity][k]
        pv = [sb(f"pv{i}", [128, 512]) for i in range(4)]
        ps = [st.enter_context(nc.psum_tensor(f"ps{i}", [128, 512], F32))
              for i in range(4)]
        at = [sb(f"at{i}", [128, 8 * 128]) for i in range(4)]
        btg = [sb(f"btg{i}", [128, 8 * 128]) for i in range(4)]
        rt = [sb(f"rt{i}", [128, SLOT_TOT]) for i in range(4)]

        s_cl = sem("s_cl")               # const loads (8 dmas -> 128)
        s_load = (sem("s_load0"), sem("s_load1"))  # embT loads, by chunk parity
        s_mm = sem("s_mm")               # matmuls (+1 each; 2 per tile)
        s_bias = sem("s_bias")           # bias adds (+1 per group)
        s_pw = tuple(sem(f"s_pw{i}") for i in range(4))  # p-write dmas, by g%4
        s_g = tuple(sem(f"s_g{i}") for i in range(4))  # gathers, by k%4 (+16, 32/call)
        s_mul = sem("s_mul")             # muls (+1 per call)
        s_red = sem("s_red")             # reduces (+1 per call)
        s_out = tuple(sem(f"s_out{i}") for i in range(4))  # res dmas, by bk%4

        CONSTS = 8 * 16  # 8 const dmas

        block = st.enter_context(nc.Block())

        # ------------------------------------------------ SP: all HWDGE DMAs
        def _sp_proj(load_chunk, sync):
            load_chunk(0)
            load_chunk(1)
            next_cq = 2
            for q, g in enumerate(GSEQ):
                # look ahead: issue loads for chunks starting within 3 groups
                while next_cq < NCHUNK and next_cq * CHUNK_T <= (q + 3) * 4 + 3:
                    load_chunk(next_cq)
                    next_cq += 1
                sync.wait_ge(s_bias, q + 1)
                tab = 0 if g < GROUPS1 else 1
                r0 = g * 512 if tab == 0 else (g - GROUPS1) * 512
                sync.dma_start(
                    out=pdst[tab][r0:r0 + 512, :].rearrange("(t p) d -> p t d", p=128),
                    in_=pv[q % 4][:].rearrange("p (t d) -> p t d", d=128),
                ).then_inc(s_pw[q % 4], 16)
            if not do_gather:
                for r in range(4):
                    sync.wait_ge(s_pw[r], 16 * len(range(r, NGROUP, 4)))

        @block.sync
        def _(sync):
            for k in range(2):
                sync.dma_start(out=w1c[:, k * 128:(k + 1) * 128],
                               in_=w1[k * 128:(k + 1) * 128, :]).then_inc(s_cl, 16)
                sync.dma_start(out=w2c[:, k * 128:(k + 1) * 128],
                               in_=w2[k * 128:(k + 1) * 128, :]).then_inc(s_cl, 16)
            sync.dma_start(out=bt[0][:], in_=b1f[:]).then_inc(s_cl, 16)
            sync.dma_start(out=bt[1][:], in_=b2f[:]).then_inc(s_cl, 16)
            sync.dma_start(out=idxt[0][:], in_=idxa[:]).then_inc(s_cl, 16)
            sync.dma_start(out=idxt[1][:], in_=idxb[:]).then_inc(s_cl, 16)

            def load_chunk(cq):
                if cq >= 2:
                    # buffer cq%2 previously held chunk cq-2; wait until consumed
                    sync.wait_ge(s_mm, 2 * CHUNK_T * (cq - 1))
                tab, col0 = _chunk_src(CSEQ[cq])
                src = e1t if tab == 0 else e2t
                par = cq % 2
                sync.dma_start(out=et[par][0][:],
                               in_=src[0:128, col0:col0 + EMB_COLS]).then_inc(s_load[par], 16)
                sync.dma_start(out=et[par][1][:],
                               in_=src[128:256, col0:col0 + EMB_COLS]).then_inc(s_load[par], 16)

            if do_proj:
                _sp_proj(load_chunk, sync)

            if not do_gather:
                return
            for bk in range(NBUCKET):
                sync.wait_ge(s_red, len(CALLS) * (bk + 1))
                sync.dma_start(out=res[bk], in_=rt[bk % 4][:]).then_inc(s_out[bk % 4], 16)
            for r in range(4):
                sync.wait_ge(s_out[r], 16 * len(range(r, NBUCKET, 4)))

        # ------------------------------------------------ PE: projections
        @block.tensor
        def _(tensor):
            if not do_proj:
                return
            tensor.wait_ge(s_cl, CONSTS)
            for q, g in enumerate(GSEQ):
                tab = 0 if g < GROUPS1 else 1
                wc = w1c if tab == 0 else w2c
                for j in range(4):
                    tq = q * 4 + j
                    cid, _, col0 = _chunk_of_tile(g * 4 + j)
                    cq = CPOS[cid]
                    if tq == cq * CHUNK_T:  # first processed tile of chunk
                        tensor.wait_ge(s_load[cq % 2], 32 * (cq // 2 + 1))
                    if j == 0 and q >= 4:
                        tensor.wait_ge(s_bias, q - 3)  # psum bank q%4 free
                    out = ps[q % 4][:, j * 128:(j + 1) * 128]
                    tensor.matmul(out=out, lhsT=et[cq % 2][0][:, col0:col0 + 128],
                                  rhs=wc[:, 0:128], start=True, stop=False).then_inc(s_mm, 1)
                    tensor.matmul(out=out, lhsT=et[cq % 2][1][:, col0:col0 + 128],
                                  rhs=wc[:, 128:256], start=False, stop=True).then_inc(s_mm, 1)

        # ------------------------------------------------ DVE: bias + dot
        @block.vector
        def _(vector):
            def emit_call(k):
                bk, ci = k // len(CALLS), k % len(CALLS)
                S = CALL_SLOTS[ci]
                scol = sum(CALL_SLOTS[:ci])
                vector.wait_ge(s_g[k % 4], 32 * (k // 4 + 1))
                if ci == 0 and bk >= 4:
                    vector.wait_ge(s_out[bk % 4], 16 * (bk // 4))  # rt[bk%4] drained
                a3 = at[k % 4][:, :S * 128]
                b3 = btg[k % 4][:, :S * 128]
                vector.tensor_mul(out=a3, in0=a3, in1=b3).then_inc(s_mul, 1)
                vector.wait_ge(s_mul, k + 1)
                vector.reduce_sum(
                    out=rt[bk % 4][:, scol:scol + S],
                    in_=at[k % 4][:, :S * 128].rearrange("p (s d) -> p s d", d=128),
                    axis=AX.X,
                ).then_inc(s_red, 1)

            vector.wait_ge(s_cl, CONSTS)
            next_k = 0
            for q, g in enumerate(GSEQ) if do_proj else ():
                vector.wait_ge(s_mm, 8 * q + 8)
                if q >= 4:
                    vector.wait_ge(s_pw[q % 4], 16 * (q // 4))  # pv[q%4] drained
                tab = 0 if g < GROUPS1 else 1
                vector.tensor_add(out=pv[q % 4][:], in0=ps[q % 4][:],
                                  in1=bt[tab][:]).then_inc(s_bias, 1)
                if do_gather and q >= INTERLEAVE_Q and next_k < NCALL:
                    emit_call(next_k)
                    next_k += 1
            while do_gather and next_k < NCALL:
                emit_call(next_k)
                next_k += 1

        # ------------------------------------------------ Pool: gathers
        @block.gpsimd
        def _(gpsimd):
            if not do_gather:
                return
            from concourse import library_config
            gpsimd.load_library(library_config.mlp)
            regs = {gsz: gpsimd.to_reg(gsz) for gsz in sorted(set(CALLS))}
            gpsimd.wait_ge(s_cl, CONSTS)
            gated_fi = -1
            for k in range(NCALL):
                bk, ci = k // len(CALLS), k % len(CALLS)
                if do_proj and ci == 0 and bk // NTB > gated_fi:
                    gated_fi = bk // NTB
                    n = FI_READY[gated_fi]
                    for r in range(4):
                        gpsimd.wait_ge(s_pw[r], 16 * len(range(r, n, 4)))
                fi, ti = bk // NTB, bk % NTB
                gsz = CALLS[ci]
                S = CALL_SLOTS[ci]
                col0 = bk * COLS_PER_BUCKET + sum(CALL_COLS[:ci])
                ncols = CALL_COLS[ci]
                if k >= 4:
                    gpsimd.wait_ge(s_red, k - 3)  # at/bt[k%4] consumed
                gpsimd.dma_gather(
                    out_ap=at[k % 4][:, :S * 128].rearrange("p (s d) -> p s d", d=128),
                    in_ap=p1d[fi * NBP:(fi + 1) * NBP, :],
                    idxs_ap=idxt[0][:, col0:col0 + ncols],
                    num_idxs=gsz, num_idxs_reg=regs[gsz], elem_size=D_OUT,
                    queue_num=0,
                ).then_inc(s_g[k % 4], 16)
                gpsimd.dma_gather(
                    out_ap=btg[k % 4][:, :S * 128].rearrange("p (s d) -> p s d", d=128),
                    in_ap=p2d[ti * NBP:(ti + 1) * NBP, :],
                    idxs_ap=idxt[1][:, col0:col0 + ncols],
                    num_idxs=gsz, num_idxs_reg=regs[gsz], elem_size=D_OUT,
                    queue_num=0,
                ).then_inc(s_g[k % 4], 16)

    return nc


_NC_CACHE = None


def _get_nc():
    global _NC_CACHE
    if _NC_CACHE is None:
        nc = build_bass()
        from concourse.library_overlay import lower_extended_insts
        lower_extended_insts(nc)
        _NC_CACHE = nc
    return _NC_CACHE


# ---------------------------------------------------------------- host side
def _marshal(emb_1, emb_2, nodes_from_to, W1, b1, W2, b2):
    """Shard/bucket inputs per core.  Returns (in_maps, bookkeeping)."""
    f = np.asarray(nodes_from_to[:, 0], dtype=np.int64)
    t = np.asarray(nodes_from_to[:, 1], dtype=np.int64)
    emb_1 = np.ascontiguousarray(np.asarray(emb_1, dtype=np.float32))
    emb_2 = np.ascontiguousarray(np.asarray(emb_2, dtype=np.float32))
    W1 = np.asarray(W1, dtype=np.float32)
    W2 = np.asarray(W2, dtype=np.float32)
    b1 = np.asarray(b1, dtype=np.float32).reshape(-1)
    b2 = np.asarray(b2, dtype=np.float32).reshape(-1)

    core = (f // (NFB * NB)) * 4 + t // (NTB * NB)
    order0 = np.argsort(core, kind="stable")
    ccnt = np.bincount(core, minlength=N_CORES)
    coff = np.concatenate([[0], np.cumsum(ccnt)])

    b1f = np.tile(b1.reshape(1, D_OUT), (128, 4)).astype(np.float32)
    b2f = np.tile(b2.reshape(1, D_OUT), (128, 4)).astype(np.float32)

    in_maps, books = [], []
    for c in range(N_CORES):
        a, b = c // 4, c % 4
        sel = order0[coff[c]:coff[c + 1]]
        fc, tcv = f[sel], t[sel]
        fi = fc // NB - NFB * a
        ti = tcv // NB - NTB * b
        fl = (fc % NB).astype(np.int16)
        tl = (tcv % NB).astype(np.int16)
        bk = fi * NTB + ti
        o2 = np.argsort(bk, kind="stable")
        sel2, fl2, tl2 = sel[o2], fl[o2], tl[o2]
        cnts = np.bincount(bk, minlength=NBUCKET)
        if (cnts > CAP).any():
            raise RuntimeError(f"bucket overflow on core {c}: {cnts}")
        pos = np.concatenate([[0], np.cumsum(cnts)])

        slots_a = np.zeros((NBUCKET, CAP), np.int16)
        slots_b = np.zeros((NBUCKET, CAP), np.int16)
        for k in range(NBUCKET):
            slots_a[k, :cnts[k]] = fl2[pos[k]:pos[k + 1]]
            slots_b[k, :cnts[k]] = tl2[pos[k]:pos[k + 1]]
        # wrap by 16: idx i of a bucket at (partition i%16, col i//16),
        # replicated across the 8 groups of 16 partitions
        wa = slots_a.reshape(NBUCKET, CAP // 16, 16).transpose(0, 2, 1)
        wb = slots_b.reshape(NBUCKET, CAP // 16, 16).transpose(0, 2, 1)
        idxa = np.concatenate([np.tile(wa[k], (8, 1)) for k in range(NBUCKET)], axis=1)
        idxb = np.concatenate([np.tile(wb[k], (8, 1)) for k in range(NBUCKET)], axis=1)

        e1t = np.zeros((D_IN, P1_ROWS), np.float32)
        for i in range(NFB):
            blk = emb_1[(NFB * a + i) * NB:(NFB * a + i + 1) * NB]
            e1t[:, i * NBP:i * NBP + NB] = blk.T
        e2t = np.zeros((D_IN, P2_ROWS), np.float32)
        for i in range(NTB):
            blk = emb_2[(NTB * b + i) * NB:(NTB * b + i + 1) * NB]
            e2t[:, i * NBP:i * NBP + NB] = blk.T

        in_maps.append({
            "e1t": e1t, "e2t": e2t,
            "w1": W1, "w2": W2, "b1f": b1f, "b2f": b2f,
            "idxa": np.ascontiguousarray(idxa),
            "idxb": np.ascontiguousarray(idxb),
        })
        books.append((sel2, cnts, pos))
    return in_maps, books


def _unmarshal(results, books, n_edges):
    out = np.empty(n_edges, np.float32)
    scol0 = np.concatenate([[0], np.cumsum(CALL_SLOTS)])
    for c in range(N_CORES):
        sel2, cnts, pos = books[c]
        r = results[c]["res"]  # [NBUCKET, 128, SLOT_TOT]
        for k in range(NBUCKET):
            if cnts[k] == 0:
                continue
            arr = r[k]
            stream = np.concatenate([
                arr[:, scol0[ci]:scol0[ci] + CALL_SLOTS[ci]].T.reshape(-1)
                for ci in range(len(CALLS))
            ])
            out[sel2[pos[k]:pos[k + 1]]] = stream[:cnts[k]]
    return out


def _run(inputs, trace=False, **run_kwargs):
    from concourse.bass_utils import run_bass_kernel_spmd

    nc = _get_nc()
    in_maps, books = _marshal(**inputs)
    r = run_bass_kernel_spmd(
        nc, in_maps, core_ids=list(range(N_CORES)), trace=trace, **run_kwargs
    )
    out = _unmarshal(r.results, books, len(inputs["nodes_from_to"]))
    return out, r


def kernel(**inputs) -> np.ndarray:
    out, _ = _run(inputs, trace=False)
    return out



# revision 23
# speedup vs baseline: 1.5064x; 1.5064x over previous
"""Trainium2 Bass kernel for nn_BetweenClusterFC (v9).

Computes out[e] = (emb_1[f[e]] @ W1 + b1) . (emb_2[t[e]] @ W2 + b2)
for E = 1.6M edges over N = 100k nodes, D_IN = 256, D_OUT = 128.

Strategy (8 NeuronCores, SPMD, full inputs in / full output out):
  - 2x4 core grid: core (a, b) handles edges with from-node in
    [50000a, 50000(a+1)) and to-node in [25000b, 25000(b+1)).
  - FROM side has NO per-edge DMA gather.  Edges are sorted by from-node
    and packed into 416 groups of <=512 edges touching <=128 distinct
    from-nodes; from-nodes are relabeled into 128-aligned windows.  Per
    128-edge tile a host-provided fp8 one-hot selector mask S^T
    [window-node, lane] turns one PE matmul  A = S^T.T @ P1w  into a row
    expansion of the SBUF-resident projected table.
  - TO side: p2 rows are projected into a DRAM table and fetched per edge
    with SWDGE dma_gather (1024-index calls, 256B rows).
  - DVE multiplies A (Act-evacuated to bf16) with the gathered B rows and
    reduces in two stages (bf16 seg-16 partials, fp32 final).
  - bf16 arithmetic throughout (fp32 PSUM accumulation + final reduce).

Everything data-dependent (masks, relabeling, gather indices) is INPUT
DATA - the Bass program is static and shared by all 8 cores.
"""

import contextlib

import numpy as np

import concourse.bass as bass
import concourse.mybir as mybir

# ---------------------------------------------------------------- constants
N_NODES = 100_000
D_IN = 256
D_OUT = 128
N_CORES = 8

FB = 50_000          # from-group nodes per core
TB = 25_000          # to-group nodes per core
TBP = 25_088         # padded to-table rows (196*128)

NGRP = 416           # from-windows (128 relabeled nodes each)
TPG = 4              # edge-tiles per window
NTILE = NGRP * TPG   # 1664
NSLOT = NTILE * 128  # 212992 edge slots
P1C = NGRP * 128     # 53248 relabeled from-node columns

GCALL = 1024                 # dma_gather idxs per call
NCALL = NSLOT // GCALL       # 208
TPC = GCALL // 128           # 8 tiles per call
IDXC = NSLOT // 16           # 13312
MPC = TPC * 128              # 1024 mask cols per call

ECH = 2048                   # embT cols per load chunk
NCHT = 13                    # e2 chunks (12*2048 + 512)
NCH1 = P1C // ECH            # 26 e1 chunks
NCHUNK = NCHT + NCH1         # 39
NW2 = TBP // 128             # 196 p2 windows
NW1 = NGRP                   # 416 p1 windows

F32 = mybir.dt.float32
BF16 = mybir.dt.bfloat16
FP8 = mybir.dt.float8e4
I16 = mybir.dt.int16
AX = mybir.AxisListType

USE_DOUBLE_ROW = False       # fp8 DoubleRow perf mode on expansion matmuls


def _chunk_cols(c):
    """chunk id -> (first window, n cols)."""
    if c < NCHT:
        return c * 16, (512 if c == NCHT - 1 else ECH)
    return NW2 + (c - NCHT) * 16, ECH


# ---------------------------------------------------------------- device code
def build_bass(with_bias=False):
    nc = bass.Bass()

    e1t = nc.dram_tensor("e1t", [D_IN, P1C], BF16, kind="ExternalInput")
    e2t = nc.dram_tensor("e2t", [D_IN, TBP], BF16, kind="ExternalInput")
    w1 = nc.dram_tensor("w1", [D_IN, D_OUT], BF16, kind="ExternalInput")
    w2 = nc.dram_tensor("w2", [D_IN, D_OUT], BF16, kind="ExternalInput")
    masks = nc.dram_tensor("masks", [NCALL // 2, 128, 2 * MPC], FP8,
                           kind="ExternalInput")
    idxb = nc.dram_tensor("idxb", [128, IDXC], I16, kind="ExternalInput")
    res = nc.dram_tensor("res", [NCALL // 4, 128, 4 * TPC], F32,
                         kind="ExternalOutput")
    if with_bias:
        b1f = nc.dram_tensor("b1f", [128, 512], BF16, kind="ExternalInput")
        b2f = nc.dram_tensor("b2f", [128, 512], BF16, kind="ExternalInput")
    p2d = nc.dram_tensor("p2d", [TBP, D_OUT], BF16, kind="Internal")

    # window -> (chunk, col offset within chunk half)
    WCH = {}
    for c in range(NCHUNK):
        w0, ncols = _chunk_cols(c)
        for j in range(ncols // 128):
            WCH[w0 + j] = (c, j * 128)

    st = contextlib.ExitStack()
    with st:
        sb = lambda nm, shape, dt=BF16: st.enter_context(nc.sbuf_tensor(nm, shape, dt))
        sem = lambda nm: st.enter_context(nc.semaphore(name=nm))

        w1c = sb("w1c", [128, 256])
        w2c = sb("w2c", [128, 256])
        idxt = sb("idxt", [128, IDXC], I16)
        p1sb = sb("p1sb", [128, P1C])
        et = [sb(f"et{k}", [128, 2 * ECH]) for k in range(2)]
        pv = [sb(f"pv{i}", [128, 1024]) for i in range(2)]
        msb = [sb(f"msb{i}", [128, 2 * MPC], FP8) for i in range(3)]
        asb = [sb(f"asb{i}", [128, 2 * GCALL]) for i in range(3)]
        bsb = [sb(f"bsb{i}", [128, 2 * GCALL]) for i in range(4)]
        mt = [sb(f"mt{i}", [128, 2 * GCALL]) for i in range(3)]
        r1 = [sb(f"r1_{i}", [128, 2 * GCALL // 16]) for i in range(2)]
        rt = [sb(f"rt{i}", [128, 4 * TPC], F32) for i in range(2)]
        if with_bias:
            bt1 = sb("bt1", [128, 512])
            bt2 = sb("bt2", [128, 512])

        pp = [st.enter_context(nc.psum_tensor(f"pp{i}", [128, 512], F32))
              for i in range(2)]
        pa = [st.enter_context(nc.psum_tensor(f"pa{i}", [128, 1024], F32))
              for i in range(3)]

        s_cl = sem("s_cl")        # const loads
        s_ech = tuple(sem(f"s_ech{i}") for i in range(4))  # chunk loads
        s_pp = sem("s_pp")        # proj window done (+1; p2 then p1 order)
        s_pev = sem("s_pev")      # p2 BANK evac (+1 per 4 windows)
        s_p2w = tuple(sem(f"s_p2w{i}") for i in range(4))  # p2d writes
        s_p1 = sem("s_p1")        # p1 BANK evac (+1 per 4 windows)
        s_mk = tuple(sem(f"s_mk{i}") for i in range(4))  # mask loads
        s_exp = sem("s_exp")      # expansion matmul (+1 per tile)
        s_aev = sem("s_aev")      # A evac (+1 per pa bank = 4 tiles)
        s_g = tuple(sem(f"s_g{i}") for i in range(8))   # gathers (+16/call)
        s_mul = sem("s_mul")      # +1 per call
        s_r1 = sem("s_r1")        # +1 per call
        s_r2 = sem("s_r2")        # +1 per call
        s_out = tuple(sem(f"s_out{i}") for i in range(4))  # res dma (+16)

        CONSTS = (5 + (2 if with_bias else 0)) * 16
        NP2W = NW2 // 4           # 49 p2d writes, one per psum bank

        block = st.enter_context(nc.Block())

        # ------------------------------------------------ SP: HWDGE DMAs
        @block.sync
        def _(sync):
            for k in range(2):
                sync.dma_start(out=w1c[:, k * 128:(k + 1) * 128],
                               in_=w1[k * 128:(k + 1) * 128, :]).then_inc(s_cl, 16)
                sync.dma_start(out=w2c[:, k * 128:(k + 1) * 128],
                               in_=w2[k * 128:(k + 1) * 128, :]).then_inc(s_cl, 16)
            sync.dma_start(out=idxt[:], in_=idxb[:]).then_inc(s_cl, 16)
            if with_bias:
                sync.dma_start(out=bt1[:], in_=b1f[:]).then_inc(s_cl, 16)
                sync.dma_start(out=bt2[:], in_=b2f[:]).then_inc(s_cl, 16)

            def load_chunk(c):
                if c >= 2:
                    w0p, ncolsp = _chunk_cols(c - 2)
                    sync.wait_ge(s_pp, w0p + ncolsp // 128)
                w0, ncols = _chunk_cols(c)
                src = e2t if c < NCHT else e1t
                col0 = c * ECH if c < NCHT else (c - NCHT) * ECH
                for k in range(2):
                    sync.dma_start(
                        out=et[c % 2][:, k * ECH:k * ECH + ncols],
                        in_=src[k * 128:(k + 1) * 128, col0:col0 + ncols],
                    ).then_inc(s_ech[c % 4], 16)

            load_chunk(0)
            load_chunk(1)
            next_c = 2

            def pump_chunks(limit):
                nonlocal next_c
                while next_c < min(limit, NCHUNK):
                    load_chunk(next_c)
                    next_c += 1

            for wr in range(NP2W):
                pump_chunks(wr // 4 + 3)
                sync.wait_ge(s_pev, wr + 1)
                sync.dma_start(
                    out=p2d[wr * 512:(wr + 1) * 512, :].rearrange(
                        "(t p) d -> p t d", p=128),
                    in_=pv[(wr // 2) % 2][:, (wr % 2) * 512:(wr % 2) * 512 + 512]
                    .rearrange("p (t d) -> p t d", d=128),
                ).then_inc(s_p2w[wr % 4], 16)

            # interleave remaining e1 chunk loads, mask loads, res drains,
            # ordered by the p1-window at which each is needed
            NPAIR = NCALL // 2
            NRES = NCALL // 4
            items = []
            for c in range(next_c, NCHUNK):
                items.append(((c - NCHT) * 16 - 32, 0, c))
            for j in range(NPAIR):
                items.append((4 * j - 24, 1, j))
            for q in range(NRES):
                items.append((8 * q + 16, 2, q))
            items.sort(key=lambda x: (x[0], x[1]))
            for _, kind, v in items:
                if kind == 0:
                    load_chunk(v)
                    next_c = v + 1
                elif kind == 1:
                    j = v
                    if j >= 3:
                        sync.wait_ge(s_exp, 16 * (j - 2))   # msb[j%3] free
                    sync.dma_start(out=msb[j % 3][:], in_=masks[j]).then_inc(
                        s_mk[j % 4], 16)
                else:
                    q = v
                    sync.wait_ge(s_r2, 2 * (q + 1))
                    sync.dma_start(out=res[q], in_=rt[q % 2][:]).then_inc(
                        s_out[q % 4], 16)
            for r in range(4):
                sync.wait_ge(s_out[r], 16 * len(range(r, NRES, 4)))

        # ------------------------------------------------ PE
        @block.tensor
        def _(tensor):
            tensor.wait_ge(s_cl, CONSTS)
            pm = mybir.MatmulPerfMode.DoubleRow if USE_DOUBLE_ROW else None

            def proj(w):
                c, cof = WCH[w]
                wc = w2c if w < NW2 else w1c
                ch = et[c % 2]
                if cof == 0:
                    tensor.wait_ge(s_ech[c % 4], 32 * (c // 4 + 1))
                if w % 4 == 0 and w >= 8:
                    pk = (w - 8) // 4            # global bank index
                    if pk < NW2 // 4:
                        tensor.wait_ge(s_pev, pk + 1)
                    else:
                        tensor.wait_ge(s_p1, pk - NW2 // 4 + 1)
                out = pp[(w // 4) % 2][:, (w % 4) * 128:(w % 4) * 128 + 128]
                tensor.matmul(out=out, lhsT=ch[:, cof:cof + 128],
                              rhs=wc[:, 0:128], start=True, stop=False)
                tensor.matmul(out=out, lhsT=ch[:, ECH + cof:ECH + cof + 128],
                              rhs=wc[:, 128:256], start=False,
                              stop=True).then_inc(s_pp, 1)

            def exp_tile(t):
                g = t // TPG
                c = t // TPC
                j = c // 2
                if t % (2 * TPC) == 0:
                    tensor.wait_ge(s_mk[j % 4], 16 * (j // 4 + 1))
                if t % TPG == 0:
                    tensor.wait_ge(s_p1, g // 4 + 1)
                if t % TPC == 0 and c >= 3:
                    tensor.wait_ge(s_aev, c - 2)              # pa[c%3] free
                out = pa[c % 3][:, (t % TPC) * 128:(t % TPC) * 128 + 128]
                tensor.matmul(
                    out=out,
                    lhsT=msb[j % 3][:, (t % (2 * TPC)) * 128:(t % (2 * TPC)) * 128 + 128],
                    rhs=p1sb[:, g * 128:g * 128 + 128],
                    start=True, stop=True, perf_mode=pm,
                ).then_inc(s_exp, 1)

            for w in range(NW2):
                proj(w)
            LAG = 4
            for w in range(NW1):
                proj(NW2 + w)
                if w >= LAG:
                    for tt in range(TPG):
                        exp_tile((w - LAG) * TPG + tt)
            for g in range(NW1 - LAG, NW1):
                for tt in range(TPG):
                    exp_tile(g * TPG + tt)

        # ------------------------------------------------ Act: p1 + A evac
        @block.scalar
        def _(scalar):
            scalar.wait_ge(s_cl, CONSTS)
            items = []
            if not with_bias:
                for k in range(NW1 // 4):
                    items.append((4 * k, 0, k))
            for c in range(NCALL):
                items.append((2 * c + 2, 1, c))
            items.sort(key=lambda x: (x[0], x[1]))
            for _, kind, v in items:
                if kind == 0:                    # p1 bank evac (4 windows)
                    k = v
                    scalar.wait_ge(s_pp, NW2 + 4 * (k + 1))
                    src = pp[(NW2 // 4 + k) % 2][:]
                    scalar.copy(out=p1sb[:, k * 512:k * 512 + 512],
                                in_=src).then_inc(s_p1, 1)
                else:                            # A evac: whole call
                    c = v
                    scalar.wait_ge(s_exp, TPC * (c + 1))
                    if c % 2 == 0 and c >= 6:
                        scalar.wait_ge(s_mul, c // 2 - 2)   # asb free
                    scalar.copy(
                        out=asb[(c // 2) % 3][:, (c % 2) * GCALL:(c % 2) * GCALL + GCALL],
                        in_=pa[c % 3][:]).then_inc(s_aev, 1)

        # ------------------------------------------------ DVE: p2 evac + dot
        @block.vector
        def _(vector):
            vector.wait_ge(s_cl, CONSTS)
            with nc.allow_low_precision(reason="bf16 stage-1 partials"):
                for k in range(NW2 // 4):        # p2 banks (4 windows each)
                    vector.wait_ge(s_pp, 4 * (k + 1))
                    if k >= 4:
                        vector.wait_ge(s_p2w[(k - 4) % 4], 16 * ((k - 4) // 4 + 1))
                    src = pp[k % 2][:]
                    dst = pv[(k // 2) % 2][:, (k % 2) * 512:(k % 2) * 512 + 512]
                    if with_bias:
                        vector.tensor_add(out=dst, in0=src,
                                          in1=bt2[:]).then_inc(s_pev, 1)
                    else:
                        vector.tensor_copy(out=dst, in_=src).then_inc(s_pev, 1)

                if with_bias:
                    for k in range(NW1 // 4):
                        vector.wait_ge(s_pp, NW2 + 4 * (k + 1))
                        src = pp[(NW2 // 4 + k) % 2][:]
                        vector.tensor_add(
                            out=p1sb[:, k * 512:k * 512 + 512], in0=src,
                            in1=bt1[:]).then_inc(s_p1, 1)

                NPAIR = NCALL // 2
                for j in range(NPAIR):
                    vector.wait_ge(s_aev, 2 * (j + 1))
                    vector.wait_ge(s_g[(2 * j) % 8], 16 * (j // 4 + 1))
                    vector.wait_ge(s_g[(2 * j + 1) % 8], 16 * (j // 4 + 1))
                    if j >= 3:
                        vector.wait_ge(s_r1, j - 2)       # mt[j%3] consumed
                    vector.tensor_mul(out=mt[j % 3][:], in0=asb[j % 3][:],
                                      in1=bsb[j % 4][:]).then_inc(s_mul, 1)
                    vector.wait_ge(s_mul, j + 1)
                    vector.reduce_sum(
                        out=r1[j % 2][:],
                        in_=mt[j % 3][:].rearrange("p (s d) -> p s d", d=16),
                        axis=AX.X).then_inc(s_r1, 1)
                    vector.wait_ge(s_r1, j + 1)
                    if j % 2 == 0 and j >= 4:
                        q = j // 2 - 2
                        vector.wait_ge(s_out[q % 4], 16 * (q // 4 + 1))
                    vector.reduce_sum(
                        out=rt[(j // 2) % 2][:, (j % 2) * 16:(j % 2) * 16 + 16],
                        in_=r1[j % 2][:].rearrange("p (s d) -> p s d", d=8),
                        axis=AX.X).then_inc(s_r2, 1)

        # ------------------------------------------------ Pool: to-gathers
        @block.gpsimd
        def _(gpsimd):
            from concourse import library_config
            gpsimd.load_library(library_config.mlp)
            gpsimd.wait_ge(s_cl, CONSTS)
            for r_ in range(4):
                gpsimd.wait_ge(s_p2w[r_], 16 * len(range(r_, NP2W, 4)))
            reg = gpsimd.to_reg(GCALL)
            for c in range(NCALL):
                if c % 2 == 0 and c >= 8:
                    gpsimd.wait_ge(s_mul, c // 2 - 3)     # bsb[(c//2)%4] free
                gpsimd.dma_gather(
                    out_ap=bsb[(c // 2) % 4][:, (c % 2) * GCALL:(c % 2) * GCALL + GCALL]
                    .rearrange("p (s d) -> p s d", d=128),
                    in_ap=p2d[:, :],
                    idxs_ap=idxt[:, c * 64:(c + 1) * 64],
                    num_idxs=GCALL, num_idxs_reg=reg, elem_size=D_OUT,
                    queue_num=0,
                ).then_inc(s_g[c % 8], 16)

    return nc


_NC_CACHE = {}


def _get_nc(with_bias):
    key = bool(with_bias)
    if key not in _NC_CACHE:
        nc = build_bass(with_bias=key)
        from concourse.library_overlay import lower_extended_insts
        lower_extended_insts(nc)
        _NC_CACHE[key] = nc
    return _NC_CACHE[key]


# ---------------------------------------------------------------- host side
def _f8(x):
    import ml_dtypes
    return np.asarray(x, dtype=ml_dtypes.float8_e4m3fn)


def _bf16(x):
    import ml_dtypes
    return np.asarray(x, dtype=ml_dtypes.bfloat16)


def _marshal(emb_1, emb_2, nodes_from_to, W1, b1, W2, b2):
    import ml_dtypes

    f = np.asarray(nodes_from_to[:, 0], dtype=np.int64)
    t = np.asarray(nodes_from_to[:, 1], dtype=np.int64)
    emb_1 = np.asarray(emb_1, dtype=np.float32)
    emb_2 = np.asarray(emb_2, dtype=np.float32)
    W1b = _bf16(np.asarray(W1, dtype=np.float32))
    W2b = _bf16(np.asarray(W2, dtype=np.float32))
    b1 = np.asarray(b1, dtype=np.float32).reshape(-1)
    b2 = np.asarray(b2, dtype=np.float32).reshape(-1)
    with_bias = bool(np.any(b1) or np.any(b2))

    core = (f // FB) * 4 + t // TB
    order0 = np.argsort(core, kind="stable")
    ccnt = np.bincount(core, minlength=N_CORES)
    coff = np.concatenate([[0], np.cumsum(ccnt)])

    one_byte = _f8(1.0).view(np.uint8)

    in_maps, books = [], []
    for ci in range(N_CORES):
        a, b_ = ci // 4, ci % 4
        sel = order0[coff[ci]:coff[ci + 1]]
        if len(sel) > NSLOT:
            raise RuntimeError(f"core {ci}: {len(sel)} edges > {NSLOT} slots")
        fl = (f[sel] - FB * a).astype(np.int64)
        tl = (t[sel] - TB * b_).astype(np.int64)

        o2 = np.argsort(fl, kind="stable")
        sel, fl, tl = sel[o2], fl[o2], tl[o2]

        # greedy grouping: <=512 edges, <=128 distinct from-nodes per group
        cnt = np.bincount(fl, minlength=FB)
        used = np.flatnonzero(cnt)
        ucnt = cnt[used]
        groups = []          # (n_edges, n_nodes)
        ge = gn = 0
        g_nodes = []
        cur_nodes = []
        for ni, ncen in zip(used, ucnt):
            if ge + ncen > 512 or gn + 1 > 128:
                groups.append((ge, gn))
                g_nodes.append(cur_nodes)
                ge = gn = 0
                cur_nodes = []
            ge += int(ncen)
            gn += 1
            cur_nodes.append(ni)
        if gn:
            groups.append((ge, gn))
            g_nodes.append(cur_nodes)
        if len(groups) > NGRP:
            raise RuntimeError(f"core {ci}: {len(groups)} groups > {NGRP}")

        # labels: node -> 128*g + rank
        label = np.full(FB, -1, np.int64)
        for g, nodes in enumerate(g_nodes):
            label[nodes] = 128 * g + np.arange(len(nodes))

        # slot assignment: group g edges -> tiles 4g..4g+3, lane-major
        edge_slot = np.empty(len(sel), np.int64)
        pos = 0
        for g, (ne, _) in enumerate(groups):
            base = g * TPG * 128
            edge_slot[pos:pos + ne] = base + np.arange(ne)
            pos += ne
        assert pos == len(sel)

        # masks (uint8 backing, fp8 bit pattern of 1.0)
        mrow = label[fl]                     # global label
        tile_of = edge_slot // 128
        lane = edge_slot % 128
        lrow = mrow - 128 * (tile_of // TPG)  # local window row 0..127
        assert (lrow >= 0).all() and (lrow < 128).all()
        mask_u8 = np.zeros((NTILE, 128, 128), np.uint8)
        mask_u8[tile_of, lrow, lane] = one_byte
        # pack per call pair: [NCALL//2, 128, 2*TPC*128]
        mask_u8 = mask_u8.reshape(NCALL // 2, 2 * TPC, 128, 128).transpose(0, 2, 1, 3)
        mask_u8 = np.ascontiguousarray(mask_u8).reshape(NCALL // 2, 128, 2 * MPC)

        # to-side gather indices, slot order, wrap 16, replicate x8
        tln = np.zeros(NSLOT, np.int16)
        tln[edge_slot] = tl.astype(np.int16)
        wa = tln.reshape(IDXC, 16).T          # [16, IDXC]
        idx_np = np.tile(wa, (8, 1))          # [128, IDXC]

        # relabeled transposed embedding tables
        e1tc = np.zeros((D_IN, P1C), ml_dtypes.bfloat16)
        lab_used = label[used]
        e1tc[:, lab_used] = _bf16(emb_1[FB * a + used].T)
        e2tc = np.zeros((D_IN, TBP), ml_dtypes.bfloat16)
        e2tc[:, :TB] = _bf16(emb_2[TB * b_:TB * (b_ + 1)].T)

        m = {
            "e1t": e1tc, "e2t": e2tc, "w1": W1b, "w2": W2b,
            "masks": mask_u8.view(ml_dtypes.float8_e4m3fn),
            "idxb": np.ascontiguousarray(idx_np),
        }
        if with_bias:
            m["b1f"] = _bf16(np.tile(b1.reshape(1, D_OUT), (128, 4)))
            m["b2f"] = _bf16(np.tile(b2.reshape(1, D_OUT), (128, 4)))
        in_maps.append(m)
        books.append((sel, edge_slot))
    return in_maps, books, with_bias


def _unmarshal(results, books, n_edges):
    out = np.empty(n_edges, np.float32)
    for ci in range(N_CORES):
        sel, edge_slot = books[ci]
        r = results[ci]["res"]               # [NCALL//4, 128, 4*TPC]
        flat = np.ascontiguousarray(r.transpose(0, 2, 1)).reshape(-1)
        out[sel] = flat[edge_slot]
    return out


def _run(inputs, trace=False, **run_kwargs):
    from concourse.bass_utils import run_bass_kernel_spmd

    in_maps, books, with_bias = _marshal(**inputs)
    nc = _get_nc(with_bias)
    r = run_bass_kernel_spmd(
        nc, in_maps, core_ids=list(range(N_CORES)), trace=trace, **run_kwargs
    )
    out = _unmarshal(r.results, books, len(inputs["nodes_from_to"]))
    return out, r


def kernel(**inputs) -> np.ndarray:
    out, _ = _run(inputs, trace=False)
    return out
